# revision 34
# baseline (speedup 1.0000x reference)
"""AVWGCN graph-conv kernel v4 for 8x Trainium2 NeuronCores (Bass/Tile).

Problem (B=32, N=4096, D=16, K=2, CIN=COUT=32):
  supports = softmax(relu(E @ E.T), axis=1)            # [N, N]
  W        = einsum('nd,dkio->nkio', E, Wp)            # per-node weights
  bias     = E @ bias_pool                             # [N, COUT]
  x_g      = stack([x, supports @ x], axis=k)          # [B, N, K, CIN]
  out      = einsum('bnki,nkio->bno', x_g, W) + bias

Sharding: row-parallel over N - each core owns NL=512 nodes (all batches),
x replicated (full m-contraction on every core). Host gathers by concat
along N. No collectives.

v4 changes vs v3 (159us):
  - batch-ordered slots: every 64-row stationary band is [u_b; x_b] with
    the same [wp1; wp0] moving rows -> ONE shared wp tensor and ONE
    output DMA per y-unit (AP [bstep,4]) instead of two
  - y-unit PSUM outputs in bf16: [128,1024] = 1 PSUM bank (was 2 fp32
    banks); ACT evac copies hit the 16-bit fast path
  - y-unit schedule 1 block behind U (was 2): tail after the last U
    block is 4 units, not 8
  - input DMA order: elr/efr/biasp -> xall[0:12] -> wp/xu -> xall[12:32]
    -> erep, with xall batched into 2 dma_starts (sync-queue relief)
  - HAM keep-warm: filler matmuls paced by the A-phase exp chain so the
    PE clock stays at 8/8 through the DMA-bound head
  - out DMAs issued from the Scalar queue (Sync queue was 60% busy)
"""

import ml_dtypes
import numpy as np

import concourse.bass as bass
import concourse.tile as tile
from concourse import bacc, mybir
from concourse.bass_utils import run_bass_kernel_spmd

BF16 = ml_dtypes.bfloat16
F16 = np.float16

B, N, D, CIN, COUT = 32, 4096, 16, 32, 32
NC = 8                  # cores
NL = N // NC            # nodes per core = 512
MC = N // 128           # m-chunks = 32
NP = MC // 2            # mc pairs = 16
BI = B * CIN            # 1024
NJ = BI // 128          # bichunks = 8 (4 batches each)
NQ = NL // 128          # n-chunks per core = 4
DO = COUT * D           # 512, free layout (d, o) with o innermost

F32 = mybir.dt.float32
BF16_DT = mybir.dt.bfloat16
FP16_DT = mybir.dt.float16

LAG = 3                 # phase-A software pipeline depth (mc chunks)


def _build_nc():
    nc = bacc.Bacc("TRN2", target_bir_lowering=False, debug=False, num_devices=NC)

    d_xall = nc.dram_tensor("xall_bi", [N, BI], BF16_DT, kind="ExternalInput").ap()
    d_xu1x = nc.dram_tensor("xu1x", [2, 32, NJ, NL], BF16_DT, kind="ExternalInput").ap()
    d_xu2x = nc.dram_tensor("xu2x", [2, 32, NJ, NL], BF16_DT, kind="ExternalInput").ap()
    d_efr = nc.dram_tensor("efr", [D, N], FP16_DT, kind="ExternalInput").ap()
    d_elr = nc.dram_tensor("elr", [D, NL], FP16_DT, kind="ExternalInput").ap()
    d_erep = nc.dram_tensor("erep", [NL, DO], FP16_DT, kind="ExternalInput").ap()
    d_wp = nc.dram_tensor("wp", [128, DO], BF16_DT, kind="ExternalInput").ap()
    d_biasp = nc.dram_tensor("biaspool", [D, COUT], FP16_DT, kind="ExternalInput").ap()
    d_out = nc.dram_tensor("out_loc", [B, NL, COUT], FP16_DT, kind="ExternalOutput").ap()

    with tile.TileContext(nc) as tc:
        with (
            tc.tile_pool(name="consts", bufs=1) as consts,
            tc.tile_pool(name="big", bufs=1) as big,
            tc.tile_pool(name="ystage", bufs=6) as ystage,
            tc.tile_pool(name="ostage", bufs=4) as ostage,
            tc.tile_pool(name="pu", bufs=2, space="PSUM") as pu_pool,
        ):
            # ---- resident SBUF tensors (DMA issue order = arrival order) ----
            sb_elr = consts.tile([D, NL], FP16_DT, tag="elr")
            nc.sync.dma_start(out=sb_elr[:], in_=d_elr)
            sb_efr = consts.tile([D, N], FP16_DT, tag="efr")
            nc.sync.dma_start(out=sb_efr[:], in_=d_efr)
            sb_biasp = consts.tile([D, COUT], FP16_DT, tag="biasp")
            nc.sync.dma_start(out=sb_biasp[:], in_=d_biasp)

            sb_xall = big.tile([128, MC, BI], BF16_DT, tag="xall")
            xall_r = d_xall.rearrange("(mc p) bi -> p mc bi", p=128)
            for mc in range(0, 12, 2):
                nc.sync.dma_start(
                    out=sb_xall[:, mc : mc + 2, :], in_=xall_r[:, mc : mc + 2, :]
                )

            sb_wp = consts.tile([128, DO], BF16_DT, tag="wp")
            nc.sync.dma_start(out=sb_wp[:], in_=d_wp)

            # xu1 rows: 0-31 u(4j+0) | 32-63 x(4j+0) | 64-95 u(4j+1) | 96-127 x(4j+1)
            # xu2 rows: 0-31 u(4j+2) | 32-63 x(4j+2) | 64-95 u(4j+3) | 96-127 x(4j+3)
            sb_xu1 = big.tile([128, NJ, NL], BF16_DT, tag="xu1")
            sb_xu2 = big.tile([128, NJ, NL], BF16_DT, tag="xu2")
            nc.sync.dma_start(out=sb_xu1[32:64, :, :], in_=d_xu1x[0])
            nc.sync.dma_start(out=sb_xu1[96:128, :, :], in_=d_xu1x[1])
            nc.sync.dma_start(out=sb_xu2[32:64, :, :], in_=d_xu2x[0])
            nc.sync.dma_start(out=sb_xu2[96:128, :, :], in_=d_xu2x[1])

            for mc in range(12, MC, 2):
                nc.sync.dma_start(
                    out=sb_xall[:, mc : mc + 2, :], in_=xall_r[:, mc : mc + 2, :]
                )

            sb_erep = consts.tile([128, NQ, DO], FP16_DT, tag="erep")
            nc.sync.dma_start(
                out=sb_erep[:], in_=d_erep.rearrange("(q p) od -> p q od", p=128)
            )

            sb_ones = consts.tile([128, 1], BF16_DT, tag="ones")
            nc.vector.memset(sb_ones[:], 1.0)
            sb_onesr = consts.tile([1, 128], F32, tag="onesr")
            nc.vector.memset(sb_onesr[:], 1.0)
            sb_bias = consts.tile([128, NQ, COUT], FP16_DT, tag="bias")
            sb_zrep = consts.tile([128, NL], F32, tag="zrep")

            sb_expA = big.tile([128, MC, NL], BF16_DT, tag="expA")
            sb_u = big.tile([128, 2, NL], BF16_DT, tag="u")  # staging ring
            sb_zst = big.tile([128, NP, NL], BF16_DT, tag="zst")

            # ---- phase A: pipelined A/exp/max + Z + U bichunks 0,1 ----
            with (
                tc.tile_pool(name="pa", bufs=4, space="PSUM") as pa_pool,
                tc.tile_pool(name="psm", bufs=1, space="PSUM") as psm_pool,
                tc.tile_pool(name="pwf", bufs=1, space="PSUM") as pwf_pool,
            ):
                # dense bf16 warmup burst: ramps the PE DVFS clock while
                # input DMAs stream; result never read
                sb_warm = ystage.tile([128, 512], BF16_DT, tag="warm", name="sb_warm")
                nc.vector.memset(sb_warm[:], 0.0)
                pwarms = [
                    pa_pool.tile([128, NL], F32, tag="pa", name=f"pwarm{w}")
                    for w in range(2)
                ]
                for w in range(10):
                    nc.tensor.matmul(
                        pwarms[w % 2][:],
                        sb_warm[:, 0:128],
                        sb_warm[:],
                        start=True,
                        stop=True,
                    )
                # filler target for the paced keep-warm matmuls
                pfill = pwf_pool.tile([128, NL], F32, tag="pwf", name="pfill")

                for q in range(NQ):
                    pb = psm_pool.tile([128, COUT], F32, tag="psm", name=f"pb{q}")
                    nc.tensor.matmul(
                        pb[:],
                        sb_elr[:, q * 128 : (q + 1) * 128],
                        sb_biasp[:],
                        start=True,
                        stop=True,
                    )
                    nc.scalar.copy(out=sb_bias[:, q, :], in_=pb[:])
                    # pre-fill the output with the bias (same for every batch);
                    # y-units then accumulate into it with DMA accum_op=add
                    dstq = d_out.rearrange("b (qq p) o -> qq p b o", p=128)[q]
                    bsrc = bass.AP(
                        tensor=sb_bias.tensor,
                        offset=sb_bias[:, q, :].offset,
                        ap=[sb_bias[:, q, :].ap[0], [0, B], [1, COUT]],
                    )
                    nc.sync.dma_start(out=dstq, in_=bsrc)

                pz = psm_pool.tile([1, NL], F32, tag="psm", name="pz")
                pu0 = pu_pool.tile([128, NL], F32, tag="pu", name="pu_0")
                pu1 = pu_pool.tile([128, NL], F32, tag="pu", name="pu_1")

                def a_stage(mc):
                    pa = pa_pool.tile([128, NL], F32, tag="pa", name=f"pa{mc}")
                    nc.tensor.matmul(
                        pa[:],
                        sb_efr[:, mc * 128 : (mc + 1) * 128],
                        sb_elr[:],
                        start=True,
                        stop=True,
                    )
                    nc.scalar.activation(
                        out=sb_expA[:, mc, :],
                        in_=pa[:],
                        func=mybir.ActivationFunctionType.Exp,
                    )
                    nc.vector.tensor_scalar_max(
                        out=sb_expA[:, mc, :], in0=sb_expA[:, mc, :], scalar1=1.0
                    )
                    # keep-warm filler: paced by this chunk's exp output so
                    # the PE sees steady work through the DMA-bound head
                    if mc % 2 == 0 and mc > 0:
                        nc.tensor.matmul(
                            pfill[:],
                            sb_warm[:, 0:128],
                            sb_expA[:, mc, :],
                            start=True,
                            stop=True,
                        )

                def zu_stage(mc):
                    for j in (0, 1):
                        nc.tensor.matmul(
                            (pu0, pu1)[j][:],
                            sb_xall[:, mc, j * 128 : (j + 1) * 128],
                            sb_expA[:, mc, :],
                            start=(mc == 0),
                            stop=(mc == MC - 1),
                        )
                    # Z: pair-add on DVE (idle in the head), halving the
                    # ones-matmul count — phase A is PE-paced
                    if mc % 2 == 1:
                        t = mc // 2
                        nc.vector.tensor_add(
                            sb_zst[:, t, :],
                            sb_expA[:, mc - 1, :],
                            sb_expA[:, mc, :],
                        )
                        nc.tensor.matmul(
                            pz[:],
                            sb_ones[:],
                            sb_zst[:, t, :],
                            start=(t == 0),
                            stop=(t == NP - 1),
                        )

                for mc in range(MC + LAG):
                    if mc < MC:
                        a_stage(mc)
                    if mc >= LAG:
                        zu_stage(mc - LAG)

                # 1/Z on [1,NL] (fast approx), then DRAM-bounce broadcast
                sb_z1 = ystage.tile([1, NL], F32, tag="z1", name="sb_z1")
                nc.scalar.copy(out=sb_z1[:], in_=pz[:])
                nc.vector.reciprocal_approx_fast(out=sb_z1[:], in_=sb_z1[:])
                pzb = pa_pool.tile([128, NL], F32, tag="pa", name="pzb")
                nc.tensor.matmul(
                    pzb[:], sb_onesr[:], sb_z1[:], start=True, stop=True
                )
                nc.scalar.copy(out=sb_zrep[:], in_=pzb[:])

            def u_evac(j, pu):
                """pu -> sb_u staging (x 1/Z), then copy u strips into xu quadrants."""
                s = j % 2
                nc.vector.tensor_mul(sb_u[:, s, :], pu[:], sb_zrep[:])
                nc.sync.dma_start(out=sb_xu1[0:32, j, :], in_=sb_u[0:32, s, :])
                nc.sync.dma_start(out=sb_xu1[64:96, j, :], in_=sb_u[32:64, s, :])
                nc.sync.dma_start(out=sb_xu2[0:32, j, :], in_=sb_u[64:96, s, :])
                nc.sync.dma_start(out=sb_xu2[64:96, j, :], in_=sb_u[96:128, s, :])

            u_evac(0, pu0)
            u_evac(1, pu1)

            # ---- U bichunks 2..7 with Y units spread between blocks ----
            py_cm = tc.tile_pool(name="py", bufs=3, space="PSUM")
            py_pool = py_cm.__enter__()

            def y_unit(q, j):
                """Final contraction for n-chunk q, batches 4j+{0,1,2,3}.
                Two 64-row fused [u_b; x_b] stationaries per xu tensor on PE
                bands (0,0)/(64,0); slots in yh are batch-ordered."""
                nsl = slice(q * 128, (q + 1) * 128)
                yh = ystage.tile([128, 4, DO], FP16_DT, tag="yh", name=f"yh{q}_{j}")
                py_a = py_pool.tile([128, 1024], F32, tag="py", name=f"pya{q}_{j}")
                py_b = py_pool.tile([128, 1024], F32, tag="py", name=f"pyb{q}_{j}")
                nc.tensor.matmul(
                    py_a[:, 0:512], sb_xu1[0:64, j, nsl], sb_wp[0:64, :],
                    start=True, stop=True,
                )
                nc.tensor.matmul(
                    py_a[:, 512:1024], sb_xu1[64:128, j, nsl], sb_wp[64:128, :],
                    start=True, stop=True,
                )
                nc.tensor.matmul(
                    py_b[:, 0:512], sb_xu2[0:64, j, nsl], sb_wp[0:64, :],
                    start=True, stop=True,
                )
                nc.tensor.matmul(
                    py_b[:, 512:1024], sb_xu2[64:128, j, nsl], sb_wp[64:128, :],
                    start=True, stop=True,
                )
                nc.scalar.copy(out=yh[:, 0:2, :], in_=py_a[:])
                nc.scalar.copy(out=yh[:, 2:4, :], in_=py_b[:])
                # multiply by E[n, d] (broadcast over slots and o)
                ebase = sb_erep[:, q, :]
                ebc = bass.AP(
                    tensor=ebase.tensor,
                    offset=ebase.offset,
                    ap=[ebase.ap[0], [0, 4], [1, DO]],
                )
                nc.vector.tensor_mul(yh[:], yh[:], ebc)
                # tree-reduce over d (outer of (d, o): contiguous halves).
                # Level 1 (the widest) runs on GPSIMD to offload the DVE,
                # which is the global bottleneck engine.
                y4 = yh[:].rearrange("p b (d o) -> p b d o", o=COUT)
                for half in (8, 4, 2, 1):
                    nc.vector.tensor_add(
                        y4[:, :, 0:half, :],
                        y4[:, :, 0:half, :],
                        y4[:, :, half : 2 * half, :],
                    )
                # slots (0,1,2,3) = batches 4j+(0,1,2,3): one accumulating DMA
                # straight from the tree output (bias was pre-filled in DRAM)
                dst = d_out.rearrange("b (q p) o -> q p b o", p=128)[q]
                bstep = dst.ap[1][0]  # element stride between batches
                dap = bass.AP(
                    tensor=dst.tensor,
                    offset=dst.offset + 4 * j * bstep,
                    ap=[dst.ap[0], [bstep, 4], [1, COUT]],
                )
                nc.gpsimd.dma_start(
                    out=dap, in_=y4[:, :, 0, :], accum_op=mybir.AluOpType.add
                )

            # units become ready as u[j] strips land; run 1 block behind.
            # At block 2 the DVE has nothing queued yet, so two units are
            # issued BEFORE the U matmuls to bridge that idle gap; the rest
            # follow u_evac so its DVE mul isn't starved (the pu ring and
            # the next block's units depend on it promptly).
            ready = [(q, j) for j in (0, 1) for q in range(NQ)]
            for j in range(2, NJ):
                if j == 2:
                    # bridge the DVE idle gap while block 2's U matmuls run
                    y_unit(*ready.pop(0))
                    y_unit(*ready.pop(0))
                pu = pu_pool.tile([128, NL], F32, tag="pu", name=f"pu_{j}")
                for mc in range(MC):
                    nc.tensor.matmul(
                        pu[:],
                        sb_xall[:, mc, j * 128 : (j + 1) * 128],
                        sb_expA[:, mc, :],
                        start=(mc == 0),
                        stop=(mc == MC - 1),
                    )
                u_evac(j, pu)
                for _ in range(min(5, len(ready))):
                    y_unit(*ready.pop(0))
                ready += [(q, j) for q in range(NQ)]
            for q, jj in ready:
                y_unit(q, jj)

            py_cm.__exit__(None, None, None)

    nc.compile()
    return nc


_CACHED = {}


def _get_nc():
    if "nc" not in _CACHED:
        _CACHED["nc"] = _build_nc()
    return _CACHED["nc"]


def _prep_inputs(x, weights_pool, bias_pool, node_embeddings):
    x = np.asarray(x, dtype=np.float32)
    wp = np.asarray(weights_pool, dtype=np.float32)
    bp = np.asarray(bias_pool, dtype=np.float32)
    E = np.asarray(node_embeddings, dtype=np.float32)

    xall = np.ascontiguousarray(x.transpose(1, 0, 2)).reshape(N, BI).astype(BF16)
    ET = np.ascontiguousarray(E.T).astype(F16)
    # wp_k[i, d*COUT+o] = Wp[d, k, i, o]  (d-outer, o-inner free layout)
    wp0 = np.ascontiguousarray(wp[:, 0].transpose(1, 0, 2)).reshape(CIN, DO)
    wp1 = np.ascontiguousarray(wp[:, 1].transpose(1, 0, 2)).reshape(CIN, DO)
    wpc = np.concatenate([wp1, wp0, wp1, wp0], axis=0).astype(BF16)

    in_maps = []
    for c in range(NC):
        loc = slice(c * NL, (c + 1) * NL)
        elocT = np.ascontiguousarray(E[loc].T).astype(F16)
        # xls[b, i, n] for local nodes
        xls = np.ascontiguousarray(x[:, loc, :].transpose(0, 2, 1)).astype(BF16)
        xu1x = np.empty((2, 32, NJ, NL), dtype=BF16)
        xu2x = np.empty((2, 32, NJ, NL), dtype=BF16)
        for j in range(NJ):
            xu1x[0, :, j, :] = xls[4 * j + 0]
            xu1x[1, :, j, :] = xls[4 * j + 1]
            xu2x[0, :, j, :] = xls[4 * j + 2]
            xu2x[1, :, j, :] = xls[4 * j + 3]
        in_maps.append(
            {
                "xall_bi": xall,
                "xu1x": xu1x,
                "xu2x": xu2x,
                "efr": ET,
                "elr": elocT,
                "erep": np.repeat(E[loc], COUT, axis=1).astype(F16),
                "wp": wpc,
                "biaspool": bp.astype(F16),
            }
        )
    return in_maps


def _run(trace=False, **inputs):
    nc = _get_nc()
    in_maps = _prep_inputs(**inputs)
    res = run_bass_kernel_spmd(nc, in_maps, core_ids=list(range(NC)), trace=trace)
    out = np.concatenate([r["out_loc"] for r in res.results], axis=1)
    return out.astype(np.float32), res


def kernel(**inputs):
    out, _ = _run(trace=False, **inputs)
    return out


def run_traced(**inputs):
    out, res = _run(trace=True, **inputs)
    return out, res
